# revision 1
# baseline (speedup 1.0000x reference)
"""Fused multi-head-free attention kernel for Trainium2, SPMD over 8 NeuronCores.

Problem: nn_Attention_2808908611625
  q = primary @ Wq + bq;  k = ctx @ Wk + bk;  v = ctx @ Wv + bv
  out = softmax(q k^T / sqrt(1024) - 1e9 * mask) @ v

Sharding: core c handles batch b = c//2, query-row half h = c%2
  (1024 query rows per core, full K/V context of its batch, K/V projection
  duplicated across the core pair).

Per-core pipeline (all matmuls bf16 with fp32 PSUM accumulation):
  1. cast-DMA (SWDGE) fp32->bf16 loads; PE-transpose primary/ctx row blocks
     so the contraction dim lands on SBUF partitions.
  2. Q/K/V projections on PE; bias folded into the PSUM->SBUF eviction
     (ACT Identity activation with per-partition bias). bv is added at the
     very end instead (softmax rows sum to 1 => attn @ (1 bv^T) = bv).
  3. S = qT.T @ kT per [128 x 512] PSUM tile; mask folded in-place with one
     DVE scalar_tensor_tensor (S += -960 * mask); P = exp(S/32) via ACT with
     accum_out producing row-sums for free. No max-subtraction: |S/32| <= ~4
     for unmasked entries and masked ones become exp(-30) ~ 1e-13.
  4. PE-transpose P tiles, PV matmul, evict with per-partition 1/rowsum
     scale, add broadcast bv, DMA out fp32.
"""

import numpy as np

import concourse.bass as bass
import concourse.mybir as mybir
import concourse.tile as tile
from concourse import bacc, bass_utils
from concourse.masks import make_identity

BF = mybir.dt.bfloat16
F32 = mybir.dt.float32
AF = mybir.ActivationFunctionType
ALU = mybir.AluOpType
AX = mybir.AxisListType

B, LQ, LKV, D = 4, 2048, 2048, 1024
P = 128
LQ_LOC = (B * LQ) // 8  # 1024 query rows per core
DC = D // P             # 8 contraction chunks
M = D // P              # 8 output-dim chunks
QT = LQ_LOC // P        # 8 query tiles per core
NT = 512                # moving free dim / psum tile width
LT = LKV // NT          # 4 kv column tiles for S
LC = LKV // P           # 16 kv chunks for PV


def build_nc():
    nc = bacc.Bacc("TRN2")

    x_d = nc.dram_tensor("primary", (LQ_LOC, D), F32, kind="ExternalInput")
    ctx_d = nc.dram_tensor("context_sequence", (LKV, D), F32, kind="ExternalInput")
    mask_d = nc.dram_tensor("mask", (LQ_LOC, LKV), F32, kind="ExternalInput")
    wq_d = nc.dram_tensor("Wq", (D, D), F32, kind="ExternalInput")
    bq_d = nc.dram_tensor("bq", (D,), F32, kind="ExternalInput")
    wk_d = nc.dram_tensor("Wk", (D, D), F32, kind="ExternalInput")
    bk_d = nc.dram_tensor("bk", (D,), F32, kind="ExternalInput")
    wv_d = nc.dram_tensor("Wv", (D, D), F32, kind="ExternalInput")
    bv_d = nc.dram_tensor("bv", (D,), F32, kind="ExternalInput")
    out_d = nc.dram_tensor("out", (LQ_LOC, D), F32, kind="ExternalOutput")

    with tile.TileContext(nc) as tc:
        with (
            tc.tile_pool(name="const", bufs=1) as const,
            tc.tile_pool(name="persist", bufs=1) as persist,
        ):
            ident = const.tile([P, P], BF)
            make_identity(nc, ident)

            # biases: b*_sb[p, m] = b[m*128 + p]
            bq_sb = const.tile([P, M], F32)
            bk_sb = const.tile([P, M], F32)
            with nc.allow_non_contiguous_dma(reason="tiny bias vectors"):
                nc.sync.dma_start(bq_sb, bq_d[:].rearrange("(m p) -> p m", p=P))
                nc.sync.dma_start(bk_sb, bk_d[:].rearrange("(m p) -> p m", p=P))

            # bv broadcast to all partitions: ones[1,128].T @ bv[1, D]
            bv_row = const.tile([1, D], BF)
            nc.gpsimd.dma_start(bv_row, bv_d[:].rearrange("(one n) -> one n", one=1))
            ones_row = const.tile([1, P], BF)
            nc.vector.memset(ones_row, 1.0)
            bv_bcast = const.tile([P, D], F32)

            qT = persist.tile([P, M, LQ_LOC], BF)   # q^T   [dattn, lq]
            kT = persist.tile([P, M, LKV], BF)      # k^T   [dattn, lkv]
            v_sb = persist.tile([P, LC, D], BF)     # v     [lkv, dout]

            # ---- phase 1a: broadcast bv via PE ----
            with tc.tile_pool(name="bvpsum", bufs=1, space="PSUM") as bvp:
                for n in range(D // NT):
                    ps = bvp.tile([P, NT], F32)
                    nc.tensor.matmul(
                        ps, ones_row, bv_row[:, bass.ts(n, NT)],
                        start=True, stop=True,
                    )
                    nc.scalar.activation(
                        bv_bcast[:, bass.ts(n, NT)], ps, AF.Copy
                    )

            # ---- phase 1b: Q projection ----
            with (
                tc.tile_pool(name="wq", bufs=1) as wqp,
                tc.tile_pool(name="xT", bufs=1) as xtp,
                tc.tile_pool(name="xstage", bufs=3) as xs,
                tc.tile_pool(name="tpsum", bufs=4, space="PSUM") as tpsum,
                tc.tile_pool(name="ppsum", bufs=2, space="PSUM") as ppsum,
            ):
                wq_sb = wqp.tile([P, DC, D], BF)
                nc.gpsimd.dma_start(
                    wq_sb, wq_d[:].rearrange("(dc p) n -> p dc n", p=P)
                )
                pT = xtp.tile([P, DC, LQ_LOC], BF)  # primary^T [din, lq]
                for rb in range(LQ_LOC // P):
                    x_sb = xs.tile([P, D], BF, tag="x")
                    nc.gpsimd.dma_start(x_sb, x_d[bass.ts(rb, P), :])
                    for dc in range(DC):
                        tp = tpsum.tile([P, P], BF, tag="tp")
                        nc.tensor.transpose(tp, x_sb[:, bass.ts(dc, P)], ident)
                        nc.vector.tensor_copy(pT[:, dc, bass.ts(rb, P)], tp)
                for m in range(M):
                    for l in range(LQ_LOC // NT):
                        ps = ppsum.tile([P, NT], F32, tag="pp")
                        for dc in range(DC):
                            nc.tensor.matmul(
                                ps,
                                wq_sb[:, dc, bass.ts(m, P)],
                                pT[:, dc, bass.ts(l, NT)],
                                start=(dc == 0), stop=(dc == DC - 1),
                            )
                        nc.scalar.activation(
                            qT[:, m, bass.ts(l, NT)], ps, AF.Identity,
                            bias=bq_sb[:, m : m + 1],
                        )

            # ---- phase 1c: K/V projections ----
            with (
                tc.tile_pool(name="wkv", bufs=1) as wkvp,
                tc.tile_pool(name="cT", bufs=1) as ctp,
                tc.tile_pool(name="xstage2", bufs=3) as xs2,
                tc.tile_pool(name="tpsum2", bufs=4, space="PSUM") as tpsum2,
                tc.tile_pool(name="ppsum2", bufs=2, space="PSUM") as ppsum2,
            ):
                wk_sb = wkvp.tile([P, DC, D], BF)
                wv_sb = wkvp.tile([P, DC, D], BF)
                nc.gpsimd.dma_start(
                    wk_sb, wk_d[:].rearrange("(dc p) n -> p dc n", p=P)
                )
                nc.gpsimd.dma_start(
                    wv_sb, wv_d[:].rearrange("(dc p) n -> p dc n", p=P)
                )
                cT = ctp.tile([P, DC, LKV], BF)  # ctx^T [din, lkv]
                for rb in range(LKV // P):
                    x_sb = xs2.tile([P, D], BF, tag="x2")
                    nc.gpsimd.dma_start(x_sb, ctx_d[bass.ts(rb, P), :])
                    for dc in range(DC):
                        tp = tpsum2.tile([P, P], BF, tag="tp2")
                        nc.tensor.transpose(tp, x_sb[:, bass.ts(dc, P)], ident)
                        nc.vector.tensor_copy(cT[:, dc, bass.ts(rb, P)], tp)
                # K^T
                for m in range(M):
                    for l in range(LT):
                        ps = ppsum2.tile([P, NT], F32, tag="pp2")
                        for dc in range(DC):
                            nc.tensor.matmul(
                                ps,
                                wk_sb[:, dc, bass.ts(m, P)],
                                cT[:, dc, bass.ts(l, NT)],
                                start=(dc == 0), stop=(dc == DC - 1),
                            )
                        nc.scalar.activation(
                            kT[:, m, bass.ts(l, NT)], ps, AF.Identity,
                            bias=bk_sb[:, m : m + 1],
                        )
                # V (natural layout), bias deferred to the end
                for lc in range(LC):
                    for n in range(D // NT):
                        ps = ppsum2.tile([P, NT], F32, tag="pp2")
                        for dc in range(DC):
                            nc.tensor.matmul(
                                ps,
                                cT[:, dc, bass.ts(lc, P)],
                                wv_sb[:, dc, bass.ts(n, NT)],
                                start=(dc == 0), stop=(dc == DC - 1),
                            )
                        nc.scalar.activation(
                            v_sb[:, lc, bass.ts(n, NT)], ps, AF.Copy
                        )

            # ---- phase 2: attention ----
            with (
                tc.tile_pool(name="mpool", bufs=2) as mpool,
                tc.tile_pool(name="epool", bufs=2) as epool,
                tc.tile_pool(name="ptpool", bufs=2) as ptpool,
                tc.tile_pool(name="rpool", bufs=4) as rpool,
                tc.tile_pool(name="opool", bufs=2) as opool,
                tc.tile_pool(name="spsum", bufs=2, space="PSUM") as spsum,
                tc.tile_pool(name="tpsum3", bufs=2, space="PSUM") as tpsum3,
                tc.tile_pool(name="avpsum", bufs=2, space="PSUM") as avpsum,
            ):
                for qt in range(QT):
                    m_sb = mpool.tile([P, LKV], BF, tag="m")
                    nc.gpsimd.dma_start(m_sb, mask_d[bass.ts(qt, P), :])
                    e_sb = epool.tile([P, LKV], BF, tag="e")
                    rs = rpool.tile([P, LT], F32, tag="rs")
                    for lt in range(LT):
                        ps = spsum.tile([P, NT], F32, tag="s")
                        for m in range(M):
                            nc.tensor.matmul(
                                ps,
                                qT[:, m, bass.ts(qt, P)],
                                kT[:, m, bass.ts(lt, NT)],
                                start=(m == 0), stop=(m == M - 1),
                            )
                        # S += -960 * mask  (=> exp((S - 960 m)/32) = P * e^-30m)
                        nc.vector.scalar_tensor_tensor(
                            ps, m_sb[:, bass.ts(lt, NT)], -960.0, ps,
                            op0=ALU.mult, op1=ALU.add,
                        )
                        nc.scalar.activation(
                            e_sb[:, bass.ts(lt, NT)], ps, AF.Exp,
                            scale=1.0 / 32.0,
                            accum_out=rs[:, lt : lt + 1],
                        )
                    rsum = rpool.tile([P, 1], F32, tag="rsum")
                    recip = rpool.tile([P, 1], F32, tag="recip")
                    nc.vector.reduce_sum(rsum, rs, axis=AX.X)
                    nc.vector.reciprocal(recip, rsum)
                    # transpose P -> [lkv, lq] chunks
                    pt_sb = ptpool.tile([P, LC, P], BF, tag="pt")
                    for lc in range(LC):
                        tp = tpsum3.tile([P, P], BF, tag="tp3")
                        nc.tensor.transpose(tp, e_sb[:, bass.ts(lc, P)], ident)
                        nc.vector.tensor_copy(pt_sb[:, lc, :], tp)
                    # out tile = (P^T)^T @ V, scaled by 1/rowsum, + bv
                    o_sb = opool.tile([P, D], F32, tag="o")
                    for n in range(D // NT):
                        ps = avpsum.tile([P, NT], F32, tag="av")
                        for lc in range(LC):
                            nc.tensor.matmul(
                                ps,
                                pt_sb[:, lc, :],
                                v_sb[:, lc, bass.ts(n, NT)],
                                start=(lc == 0), stop=(lc == LC - 1),
                            )
                        nc.scalar.activation(
                            o_sb[:, bass.ts(n, NT)], ps, AF.Identity,
                            scale=recip[:, 0:1],
                        )
                    nc.vector.tensor_add(o_sb, o_sb, bv_bcast)
                    nc.sync.dma_start(out_d[bass.ts(qt, P), :], o_sb)

    nc.finalize()
    return nc


_NC_CACHE = None


def kernel(**inputs: np.ndarray) -> np.ndarray:
    global _NC_CACHE
    if _NC_CACHE is None:
        _NC_CACHE = build_nc()
    nc = _NC_CACHE

    primary = np.ascontiguousarray(np.asarray(inputs["primary"], dtype=np.float32))
    ctx = np.ascontiguousarray(
        np.asarray(inputs["context_sequence"], dtype=np.float32)
    )
    mask = np.ascontiguousarray(np.asarray(inputs["mask"], dtype=np.float32))
    shared = {
        k: np.ascontiguousarray(np.asarray(inputs[k], dtype=np.float32))
        for k in ("Wq", "bq", "Wk", "bk", "Wv", "bv")
    }

    H = LQ // 2  # 1024
    in_maps = []
    for c in range(8):
        b, h = c // 2, c % 2
        in_maps.append(
            {
                "primary": primary[b, h * H : (h + 1) * H, :],
                "context_sequence": ctx[b],
                "mask": mask[b, h * H : (h + 1) * H, :],
                **shared,
            }
        )

    res = bass_utils.run_bass_kernel_spmd(nc, in_maps, core_ids=list(range(8)))

    out = np.empty((B, LQ, D), dtype=np.float32)
    for c in range(8):
        b, h = c // 2, c % 2
        out[b, h * H : (h + 1) * H, :] = res.results[c]["out"]
    return out


if __name__ == "__main__":
    rng = np.random.default_rng(0)
    ins = {
        "primary": rng.standard_normal((B, LQ, D), dtype=np.float32),
        "context_sequence": rng.standard_normal((B, LKV, D), dtype=np.float32),
        "mask": rng.integers(0, 2, (B, LQ, LKV)).astype(np.float32),
        "Wq": rng.uniform(-1 / 32, 1 / 32, (D, D)).astype(np.float32),
        "bq": rng.uniform(-1 / 32, 1 / 32, (D,)).astype(np.float32),
        "Wk": rng.uniform(-1 / 32, 1 / 32, (D, D)).astype(np.float32),
        "bk": rng.uniform(-1 / 32, 1 / 32, (D,)).astype(np.float32),
        "Wv": rng.uniform(-1 / 32, 1 / 32, (D, D)).astype(np.float32),
        "bv": rng.uniform(-1 / 32, 1 / 32, (D,)).astype(np.float32),
    }
    out = kernel(**ins)
    print("out", out.shape, out.dtype, float(np.abs(out).mean()))


# revision 4
# speedup vs baseline: 17265.7971x; 17265.7971x over previous
"""Fused multi-head-free attention kernel for Trainium2, SPMD over 8 NeuronCores.

Problem: nn_Attention_2808908611625
  q = primary @ Wq + bq;  k = ctx @ Wk + bk;  v = ctx @ Wv + bv
  out = softmax(q k^T / sqrt(1024) - 1e9 * mask) @ v

Sharding: core c handles batch b = c//2, query-row half h = c%2
  (1024 query rows per core, full K/V context of its batch, K/V projection
  duplicated across the core pair).

Per-core pipeline (all matmuls bf16 with fp32 PSUM accumulation):
  1. cast-DMA (SWDGE) fp32->bf16 loads; PE-transpose primary/ctx row blocks
     so the contraction dim lands on SBUF partitions.
  2. Q/K/V projections on PE; bias folded into the PSUM->SBUF eviction
     (ACT Identity activation with per-partition bias). bv is added at the
     very end instead (softmax rows sum to 1 => attn @ (1 bv^T) = bv).
  3. S = qT.T @ kT per [128 x 512] PSUM tile; mask folded in-place with one
     DVE scalar_tensor_tensor (S += -960 * mask); P = exp(S/32) via ACT with
     accum_out producing row-sums for free. No max-subtraction: |S/32| <= ~4
     for unmasked entries and masked ones become exp(-30) ~ 1e-13.
  4. PE-transpose P tiles, PV matmul, evict with per-partition 1/rowsum
     scale, add broadcast bv, DMA out fp32.
"""

import numpy as np

import concourse.bass as bass
import concourse.mybir as mybir
import concourse.tile as tile
from concourse import bacc, bass_utils
from concourse.masks import make_identity

BF = mybir.dt.bfloat16
F32 = mybir.dt.float32
AF = mybir.ActivationFunctionType
ALU = mybir.AluOpType
AX = mybir.AxisListType

B, LQ, LKV, D = 4, 2048, 2048, 1024
P = 128
LQ_LOC = (B * LQ) // 8  # 1024 query rows per core
DC = D // P             # 8 contraction chunks
M = D // P              # 8 output-dim chunks
QT = LQ_LOC // P        # 8 query tiles per core
NT = 512                # moving free dim / psum tile width
LT = LKV // NT          # 4 kv column tiles for S
LC = LKV // P           # 16 kv chunks for PV


def build_nc(reps: int = 1):
    nc = bacc.Bacc("TRN2")

    x_d = nc.dram_tensor("primary", (LQ_LOC, D), F32, kind="ExternalInput")
    ctx_d = nc.dram_tensor("context_sequence", (LKV, D), F32, kind="ExternalInput")
    mask_d = nc.dram_tensor("mask", (LQ_LOC, LKV), F32, kind="ExternalInput")
    wq_d = nc.dram_tensor("Wq", (D, D), F32, kind="ExternalInput")
    bq_d = nc.dram_tensor("bq", (D,), F32, kind="ExternalInput")
    wk_d = nc.dram_tensor("Wk", (D, D), F32, kind="ExternalInput")
    bk_d = nc.dram_tensor("bk", (D,), F32, kind="ExternalInput")
    wv_d = nc.dram_tensor("Wv", (D, D), F32, kind="ExternalInput")
    bv_d = nc.dram_tensor("bv", (D,), F32, kind="ExternalInput")
    out_d = nc.dram_tensor("out", (LQ_LOC, D), F32, kind="ExternalOutput")

    with tile.TileContext(nc) as tc:
        with (
            tc.tile_pool(name="const", bufs=1) as const,
            tc.tile_pool(name="persist", bufs=1) as persist,
        ):
            ident = const.tile([P, P], BF)
            make_identity(nc, ident)

            # biases: b*_sb[p, m] = b[m*128 + p]
            bq_sb = const.tile([P, M], F32)
            bk_sb = const.tile([P, M], F32)
            with nc.allow_non_contiguous_dma(reason="tiny bias vectors"):
                nc.sync.dma_start(bq_sb, bq_d[:].rearrange("(m p) -> p m", p=P))
                nc.sync.dma_start(bk_sb, bk_d[:].rearrange("(m p) -> p m", p=P))

            # bv broadcast to all partitions: ones[1,128].T @ bv[1, D]
            bv_row = const.tile([1, D], BF)
            nc.gpsimd.dma_start(bv_row, bv_d[:].rearrange("(one n) -> one n", one=1))
            ones_row = const.tile([1, P], BF)
            nc.vector.memset(ones_row, 1.0)
            bv_bcast = const.tile([P, D], F32)

            qT = persist.tile([P, M, LQ_LOC], BF)   # q^T   [dattn, lq]
            kT = persist.tile([P, M, LKV], BF)      # k^T   [dattn, lkv]
            v_sb = persist.tile([P, LC, D], BF)     # v     [lkv, dout]

            if reps > 1:
                loop_ctx = tc.For_i(0, reps, 1)
                loop_ctx.__enter__()

            # ---- phase 1a: broadcast bv via PE ----
            with tc.tile_pool(name="bvpsum", bufs=1, space="PSUM") as bvp:
                for n in range(D // NT):
                    ps = bvp.tile([P, NT], F32)
                    nc.tensor.matmul(
                        ps, ones_row, bv_row[:, bass.ts(n, NT)],
                        start=True, stop=True,
                    )
                    nc.scalar.activation(
                        bv_bcast[:, bass.ts(n, NT)], ps, AF.Copy
                    )

            # ---- phase 1b: Q projection ----
            with (
                tc.tile_pool(name="wq", bufs=1) as wqp,
                tc.tile_pool(name="xT", bufs=1) as xtp,
                tc.tile_pool(name="xstage", bufs=3) as xs,
                tc.tile_pool(name="tpsum", bufs=4, space="PSUM") as tpsum,
                tc.tile_pool(name="ppsum", bufs=2, space="PSUM") as ppsum,
            ):
                wq_sb = wqp.tile([P, DC, D], BF)
                nc.gpsimd.dma_start(
                    wq_sb, wq_d[:].rearrange("(dc p) n -> p dc n", p=P)
                )
                pT = xtp.tile([P, DC, LQ_LOC], BF)  # primary^T [din, lq]
                for rb in range(LQ_LOC // P):
                    x_sb = xs.tile([P, D], BF, tag="x")
                    nc.gpsimd.dma_start(x_sb, x_d[bass.ts(rb, P), :])
                    for dc in range(DC):
                        tp = tpsum.tile([P, P], BF, tag="tp")
                        nc.tensor.transpose(tp, x_sb[:, bass.ts(dc, P)], ident)
                        nc.vector.tensor_copy(pT[:, dc, bass.ts(rb, P)], tp)
                for m in range(M):
                    for l in range(LQ_LOC // NT):
                        ps = ppsum.tile([P, NT], F32, tag="pp")
                        for dc in range(DC):
                            nc.tensor.matmul(
                                ps,
                                wq_sb[:, dc, bass.ts(m, P)],
                                pT[:, dc, bass.ts(l, NT)],
                                start=(dc == 0), stop=(dc == DC - 1),
                            )
                        nc.scalar.activation(
                            qT[:, m, bass.ts(l, NT)], ps, AF.Identity,
                            bias=bq_sb[:, m : m + 1],
                        )

            # ---- phase 1c: K/V projections ----
            with (
                tc.tile_pool(name="wkv", bufs=1) as wkvp,
                tc.tile_pool(name="cT", bufs=1) as ctp,
                tc.tile_pool(name="xstage2", bufs=3) as xs2,
                tc.tile_pool(name="tpsum2", bufs=4, space="PSUM") as tpsum2,
                tc.tile_pool(name="ppsum2", bufs=2, space="PSUM") as ppsum2,
            ):
                wk_sb = wkvp.tile([P, DC, D], BF)
                wv_sb = wkvp.tile([P, DC, D], BF)
                nc.gpsimd.dma_start(
                    wk_sb, wk_d[:].rearrange("(dc p) n -> p dc n", p=P)
                )
                nc.gpsimd.dma_start(
                    wv_sb, wv_d[:].rearrange("(dc p) n -> p dc n", p=P)
                )
                cT = ctp.tile([P, DC, LKV], BF)  # ctx^T [din, lkv]
                for rb in range(LKV // P):
                    x_sb = xs2.tile([P, D], BF, tag="x2")
                    nc.gpsimd.dma_start(x_sb, ctx_d[bass.ts(rb, P), :])
                    for dc in range(DC):
                        tp = tpsum2.tile([P, P], BF, tag="tp2")
                        nc.tensor.transpose(tp, x_sb[:, bass.ts(dc, P)], ident)
                        nc.vector.tensor_copy(cT[:, dc, bass.ts(rb, P)], tp)
                # K^T
                for m in range(M):
                    for l in range(LT):
                        ps = ppsum2.tile([P, NT], F32, tag="pp2")
                        for dc in range(DC):
                            nc.tensor.matmul(
                                ps,
                                wk_sb[:, dc, bass.ts(m, P)],
                                cT[:, dc, bass.ts(l, NT)],
                                start=(dc == 0), stop=(dc == DC - 1),
                            )
                        nc.scalar.activation(
                            kT[:, m, bass.ts(l, NT)], ps, AF.Identity,
                            bias=bk_sb[:, m : m + 1],
                        )
                # V (natural layout), bias deferred to the end
                for lc in range(LC):
                    for n in range(D // NT):
                        ps = ppsum2.tile([P, NT], F32, tag="pp2")
                        for dc in range(DC):
                            nc.tensor.matmul(
                                ps,
                                cT[:, dc, bass.ts(lc, P)],
                                wv_sb[:, dc, bass.ts(n, NT)],
                                start=(dc == 0), stop=(dc == DC - 1),
                            )
                        nc.scalar.activation(
                            v_sb[:, lc, bass.ts(n, NT)], ps, AF.Copy
                        )

            # ---- phase 2: attention ----
            with (
                tc.tile_pool(name="mpool", bufs=2) as mpool,
                tc.tile_pool(name="epool", bufs=2) as epool,
                tc.tile_pool(name="ptpool", bufs=2) as ptpool,
                tc.tile_pool(name="rpool", bufs=4) as rpool,
                tc.tile_pool(name="opool", bufs=2) as opool,
                tc.tile_pool(name="spsum", bufs=2, space="PSUM") as spsum,
                tc.tile_pool(name="tpsum3", bufs=2, space="PSUM") as tpsum3,
                tc.tile_pool(name="avpsum", bufs=2, space="PSUM") as avpsum,
            ):
                for qt in range(QT):
                    m_sb = mpool.tile([P, LKV], BF, tag="m")
                    nc.gpsimd.dma_start(m_sb, mask_d[bass.ts(qt, P), :])
                    e_sb = epool.tile([P, LKV], BF, tag="e")
                    rs = rpool.tile([P, LT], F32, tag="rs")
                    for lt in range(LT):
                        ps = spsum.tile([P, NT], F32, tag="s")
                        for m in range(M):
                            nc.tensor.matmul(
                                ps,
                                qT[:, m, bass.ts(qt, P)],
                                kT[:, m, bass.ts(lt, NT)],
                                start=(m == 0), stop=(m == M - 1),
                            )
                        # S += -960 * mask  (=> exp((S - 960 m)/32) = P * e^-30m)
                        nc.vector.scalar_tensor_tensor(
                            ps, m_sb[:, bass.ts(lt, NT)], -960.0, ps,
                            op0=ALU.mult, op1=ALU.add,
                        )
                        nc.scalar.activation(
                            e_sb[:, bass.ts(lt, NT)], ps, AF.Exp,
                            scale=1.0 / 32.0,
                            accum_out=rs[:, lt : lt + 1],
                        )
                    rsum = rpool.tile([P, 1], F32, tag="rsum")
                    recip = rpool.tile([P, 1], F32, tag="recip")
                    nc.vector.reduce_sum(rsum, rs, axis=AX.X)
                    nc.vector.reciprocal(recip, rsum)
                    # transpose P -> [lkv, lq] chunks
                    pt_sb = ptpool.tile([P, LC, P], BF, tag="pt")
                    for lc in range(LC):
                        tp = tpsum3.tile([P, P], BF, tag="tp3")
                        nc.tensor.transpose(tp, e_sb[:, bass.ts(lc, P)], ident)
                        nc.vector.tensor_copy(pt_sb[:, lc, :], tp)
                    # out tile = (P^T)^T @ V, scaled by 1/rowsum, + bv
                    o_sb = opool.tile([P, D], F32, tag="o")
                    for n in range(D // NT):
                        ps = avpsum.tile([P, NT], F32, tag="av")
                        for lc in range(LC):
                            nc.tensor.matmul(
                                ps,
                                pt_sb[:, lc, :],
                                v_sb[:, lc, bass.ts(n, NT)],
                                start=(lc == 0), stop=(lc == LC - 1),
                            )
                        nc.scalar.activation(
                            o_sb[:, bass.ts(n, NT)], ps, AF.Identity,
                            scale=recip[:, 0:1],
                        )
                    nc.vector.tensor_add(o_sb, o_sb, bv_bcast)
                    nc.sync.dma_start(out_d[bass.ts(qt, P), :], o_sb)

            if reps > 1:
                loop_ctx.__exit__(None, None, None)

    nc.finalize()
    return nc


_NC_CACHE = None


def kernel(**inputs: np.ndarray) -> np.ndarray:
    global _NC_CACHE
    if _NC_CACHE is None:
        _NC_CACHE = build_nc()
    nc = _NC_CACHE

    primary = np.ascontiguousarray(np.asarray(inputs["primary"], dtype=np.float32))
    ctx = np.ascontiguousarray(
        np.asarray(inputs["context_sequence"], dtype=np.float32)
    )
    mask = np.ascontiguousarray(np.asarray(inputs["mask"], dtype=np.float32))
    shared = {
        k: np.ascontiguousarray(np.asarray(inputs[k], dtype=np.float32))
        for k in ("Wq", "bq", "Wk", "bk", "Wv", "bv")
    }

    H = LQ // 2  # 1024
    in_maps = []
    for c in range(8):
        b, h = c // 2, c % 2
        in_maps.append(
            {
                "primary": primary[b, h * H : (h + 1) * H, :],
                "context_sequence": ctx[b],
                "mask": mask[b, h * H : (h + 1) * H, :],
                **shared,
            }
        )

    res = bass_utils.run_bass_kernel_spmd(nc, in_maps, core_ids=list(range(8)))

    out = np.empty((B, LQ, D), dtype=np.float32)
    for c in range(8):
        b, h = c // 2, c % 2
        out[b, h * H : (h + 1) * H, :] = res.results[c]["out"]
    return out


if __name__ == "__main__":
    rng = np.random.default_rng(0)
    ins = {
        "primary": rng.standard_normal((B, LQ, D), dtype=np.float32),
        "context_sequence": rng.standard_normal((B, LKV, D), dtype=np.float32),
        "mask": rng.integers(0, 2, (B, LQ, LKV)).astype(np.float32),
        "Wq": rng.uniform(-1 / 32, 1 / 32, (D, D)).astype(np.float32),
        "bq": rng.uniform(-1 / 32, 1 / 32, (D,)).astype(np.float32),
        "Wk": rng.uniform(-1 / 32, 1 / 32, (D, D)).astype(np.float32),
        "bk": rng.uniform(-1 / 32, 1 / 32, (D,)).astype(np.float32),
        "Wv": rng.uniform(-1 / 32, 1 / 32, (D, D)).astype(np.float32),
        "bv": rng.uniform(-1 / 32, 1 / 32, (D,)).astype(np.float32),
    }
    out = kernel(**ins)
    print("out", out.shape, out.dtype, float(np.abs(out).mean()))
